# revision 6
# baseline (speedup 1.0000x reference)
"""LSTMCell on 8 Trainium2 NeuronCores, data-parallel over the batch.

Full inputs: x/h_t/c_t [65536,128] f32, 8 gate weight matrices [128,128],
4 biases [128]. Returns (h_new, c_new) as [65536,128] f32 each.

Per core (8192 rows): slabs of 1024 rows where partition p holds DRAM rows
r0+8p..r0+8p+7 (4KB contiguous per partition per DMA -> large packets).
Load and store use the same row permutation, so the math is unaffected.

Per slab (8 tiles of 128 rows, 2 PSUM quads of 4 tiles):
  - PE transposes x/h tiles (f32r, 1.5 cyc/row) into the quad PSUM banks,
    DVE copies them to SBUF as f32r.
  - Two f32r matmuls per tile accumulate gates [128 batch, 512] into one
    PSUM bank; 4 tiles share a [128,2048] 4-bank quad.
  - Gate order [i, f, o, g]: ACT does Sigmoid over the i/f/o columns and
    Tanh over the g columns (two strided instructions, one pass over PSUM).
  - Pool (idle otherwise) does ig = i*g, fc = f*c, hn = o*tanh(c_new);
    DVE does the PSUM cast copy and c_new = ig + fc; ACT does tanh(c_new).
"""
import numpy as np
from contextlib import ExitStack

import concourse.bass as bass
import concourse.tile as tile
from concourse import bacc, mybir
from concourse.bass_utils import run_bass_kernel_spmd
from concourse.masks import make_identity

F32 = mybir.dt.float32
F32R = mybir.dt.float32r
AF = mybir.ActivationFunctionType
ALU = mybir.AluOpType

NCORES = 8
BC = 8192            # batch rows per core
RPP = 8              # DRAM rows per partition per slab (4KB contiguous)
SLAB_ROWS = 128 * RPP   # 1024
NSLAB = BC // SLAB_ROWS  # 8
NT = 4               # tiles per PSUM quad
NQ = SLAB_ROWS // (128 * NT)  # quads per slab = 2

_CACHE = {}


def _build(has_bias: bool):
    nc = bacc.Bacc("TRN2", target_bir_lowering=False, debug=False)
    x = nc.dram_tensor("x", [BC, 128], F32, kind="ExternalInput").ap()
    h = nc.dram_tensor("h", [BC, 128], F32, kind="ExternalInput").ap()
    c = nc.dram_tensor("c", [BC, 128], F32, kind="ExternalInput").ap()
    wxt = nc.dram_tensor("wxt", [128, 512], F32R, kind="ExternalInput").ap()
    wht = nc.dram_tensor("wht", [128, 512], F32R, kind="ExternalInput").ap()
    if has_bias:
        bias = nc.dram_tensor("bias", [1, 512], F32R, kind="ExternalInput").ap()
    hn = nc.dram_tensor("hn", [BC, 128], F32, kind="ExternalOutput").ap()
    cn = nc.dram_tensor("cn", [BC, 128], F32, kind="ExternalOutput").ap()

    def load_slab(sb_t, dram, r0):
        nc.sync.dma_start(
            sb_t[:].rearrange("p (r f) -> p r f", r=RPP),
            dram[r0:r0 + SLAB_ROWS, :].rearrange("(p r) f -> p r f", p=128))

    def store_slab(dram, sb_t, r0):
        nc.sync.dma_start(
            dram[r0:r0 + SLAB_ROWS, :].rearrange("(p r) f -> p r f", p=128),
            sb_t[:].rearrange("p (r f) -> p r f", r=RPP))

    with tile.TileContext(nc) as tc:
        with ExitStack() as ctx:
            const = ctx.enter_context(tc.tile_pool(name="const", bufs=1))
            inp = ctx.enter_context(tc.tile_pool(name="inp", bufs=3))
            xht = ctx.enter_context(tc.tile_pool(name="xht", bufs=3))
            qp = ctx.enter_context(tc.tile_pool(name="qp", bufs=2, space="PSUM"))
            sp = ctx.enter_context(tc.tile_pool(name="sp", bufs=3))
            op = ctx.enter_context(tc.tile_pool(name="op", bufs=2))
            tmp = ctx.enter_context(tc.tile_pool(name="tmp", bufs=3))

            ident = const.tile([128, 128], F32)
            make_identity(nc, ident)
            wx_sb = const.tile([128, 512], F32R)
            nc.sync.dma_start(wx_sb[:], wxt)
            wh_sb = const.tile([128, 512], F32R)
            nc.sync.dma_start(wh_sb[:], wht)
            if has_bias:
                ones = const.tile([1, 128], F32R)
                nc.vector.memset(ones[:], 1.0)
                b_sb = const.tile([1, 512], F32R)
                nc.sync.dma_start(b_sb[:], bias)

            warm = qp.tile([128, 2048], F32, name="warm", tag="quad")
            for _ in range(16):
                nc.tensor.matmul(warm[:, 0:128], ident[:], ident[:],
                                 is_transpose=True, start=True, stop=True)

            identr = ident[:].bitcast(F32R)
            slabs = {}

            def issue_loads(s):
                r0 = s * SLAB_ROWS
                xsl = inp.tile([128, SLAB_ROWS], F32, name=f"xsl{s}", tag="xg")
                hsl = inp.tile([128, SLAB_ROWS], F32, name=f"hsl{s}", tag="hg")
                csl = inp.tile([128, SLAB_ROWS], F32, name=f"csl{s}", tag="cg")
                for sb_t, dram in ((xsl, x), (hsl, h), (csl, c)):
                    load_slab(sb_t, dram, r0)
                slabs[s] = (xsl, hsl, csl)

            issue_loads(0)
            for s in range(NSLAB):
                if s + 1 < NSLAB:
                    issue_loads(s + 1)
                xsl, hsl, csl = slabs.pop(s)
                r0 = s * SLAB_ROWS
                hn_sl = op.tile([128, SLAB_ROWS], F32, name=f"hn{s}", tag="hn")
                cn_sl = op.tile([128, SLAB_ROWS], F32, name=f"cn{s}", tag="cn")
                for q in range(NQ):
                    goff = q * NT * 128
                    quad = qp.tile([128, 2048], F32, name=f"quad{s}_{q}",
                                   tag="quad")
                    # pass A: transposes of x/h tiles (f32r) back-to-back,
                    # then ONE wide cast over all 4 banks (strided 3D AP)
                    for t in range(NT):
                        col = t * 512
                        fs = goff + t * 128
                        nc.tensor.matmul(
                            quad[:, col:col + 128],
                            xsl[:, fs:fs + 128], ident[:],
                            is_transpose=True, start=True, stop=False)
                        nc.tensor.matmul(
                            quad[:, col + 128:col + 256],
                            hsl[:, fs:fs + 128], ident[:],
                            is_transpose=True, start=False, stop=True)
                    xh_w = xht.tile([128, 1024], F32R, name=f"xh{s}_{q}",
                                    tag="xh")
                    nc.vector.tensor_copy(
                        xh_w[:].rearrange("p (t x) -> p t x", t=NT),
                        quad[:].rearrange("p (t x) -> p t x", t=NT)[:, :, 0:256])
                    # pass B: gates matmuls = [x h] @ [WxT; WhT] (+ bias)
                    for t in range(NT):
                        col = t * 512
                        xh = xh_w[:, t * 256:(t + 1) * 256]
                        first = True
                        if has_bias:
                            nc.tensor.matmul(quad[:, col:col + 512], ones[:],
                                             b_sb[:], start=True, stop=False)
                            first = False
                        nc.tensor.matmul(quad[:, col:col + 512], xh[:, 0:128],
                                         wx_sb[:], start=first, stop=False)
                        nc.tensor.matmul(quad[:, col:col + 512], xh[:, 128:256],
                                         wh_sb[:], start=False, stop=True)

                    sig = sp.tile([128, 2048], F32, name=f"sig{s}_{q}",
                                  tag="sig")
                    quad3 = quad[:].rearrange("p (t x) -> p t x", t=NT)
                    sig3 = sig[:].rearrange("p (t x) -> p t x", t=NT)
                    # one pass over PSUM: sigmoid for i/f/o, tanh for g
                    nc.scalar.activation(sig3[:, :, 0:384], quad3[:, :, 0:384],
                                         AF.Sigmoid)
                    nc.scalar.activation(sig3[:, :, 384:512],
                                         quad3[:, :, 384:512], AF.Tanh)
                    i_ap = sig3[:, :, 0:128]
                    f_ap = sig3[:, :, 128:256]
                    o_ap = sig3[:, :, 256:384]
                    g_ap = sig3[:, :, 384:512]
                    c3 = csl[:, goff:goff + 512].rearrange(
                        "p (t x) -> p t x", t=NT)
                    ig = tmp.tile([128, 512], F32, name=f"ig{s}_{q}", tag="ig")
                    ig3 = ig[:].rearrange("p (t x) -> p t x", t=NT)
                    nc.gpsimd.tensor_mul(ig3, i_ap, g_ap)
                    fc = tmp.tile([128, 512], F32, name=f"fc{s}_{q}", tag="fc")
                    fc3 = fc[:].rearrange("p (t x) -> p t x", t=NT)
                    nc.gpsimd.tensor_mul(fc3, f_ap, c3)
                    cn_g = cn_sl[:, q * 512:(q + 1) * 512]
                    nc.vector.tensor_add(cn_g, ig[:], fc[:])
                    tc_g = tmp.tile([128, 512], F32, name=f"tc{s}_{q}",
                                    tag="tcg")
                    nc.scalar.activation(tc_g[:], cn_g, AF.Tanh)
                    tc3 = tc_g[:].rearrange("p (t x) -> p t x", t=NT)
                    hn3 = hn_sl[:, q * 512:(q + 1) * 512].rearrange(
                        "p (t x) -> p t x", t=NT)
                    nc.gpsimd.tensor_mul(hn3, o_ap, tc3)
                for sb_t, dram in ((hn_sl, hn), (cn_sl, cn)):
                    store_slab(dram, sb_t, r0)
    nc.compile()
    return nc


def _run(inputs, trace=False, tmpdir=None):
    x = np.ascontiguousarray(inputs["x"], dtype=np.float32)
    h = np.ascontiguousarray(inputs["h_t"], dtype=np.float32)
    c = np.ascontiguousarray(inputs["c_t"], dtype=np.float32)
    # gate order [i, f, o, g]
    wx = np.concatenate([inputs["W_ii"], inputs["W_if"], inputs["W_io"],
                         inputs["W_ig"]], axis=0)
    wh = np.concatenate([inputs["W_hi"], inputs["W_hf"], inputs["W_ho"],
                         inputs["W_hg"]], axis=0)
    b = np.concatenate([inputs["b_i"], inputs["b_f"], inputs["b_o"],
                        inputs["b_g"]], axis=0)
    wxt = np.ascontiguousarray(wx.T, dtype=np.float32)
    wht = np.ascontiguousarray(wh.T, dtype=np.float32)
    has_bias = bool(np.any(b))

    key = has_bias
    if key not in _CACHE:
        _CACHE[key] = _build(has_bias)
    nc = _CACHE[key]

    in_maps = []
    for i in range(NCORES):
        m = {
            "x": x[i * BC:(i + 1) * BC],
            "h": h[i * BC:(i + 1) * BC],
            "c": c[i * BC:(i + 1) * BC],
            "wxt": wxt,
            "wht": wht,
        }
        if has_bias:
            m["bias"] = b.reshape(1, 512).astype(np.float32)
        in_maps.append(m)

    res = run_bass_kernel_spmd(nc, in_maps, core_ids=list(range(NCORES)),
                               trace=trace, tmpdir=tmpdir)
    h_new = np.concatenate([r["hn"] for r in res.results], axis=0)
    c_new = np.concatenate([r["cn"] for r in res.results], axis=0)
    return h_new, c_new, res


def kernel(**inputs):
    h_new, c_new, _ = _run(inputs, trace=False)
    return h_new, c_new
